# revision 30
# baseline (speedup 1.0000x reference)
"""BoundaryLoss kernel for 8 Trainium2 NeuronCores.

Math (equivalent to the reference):
  boundary b(i,j) = [L(i,j-1) != L(i,j+1)]_clamped OR [L(i-1,j) != L(i+1,j)]_clamped
    (union of class-1/2 indicator boundaries == "any label change", since any
     differing pair in {0,1,2} differs in membership of class 1 or class 2)
  ce  = logsumexp_c(x) - x[label]
  loss = sum(ce * b) / (sum(b) + 1e-8)

Decomposition used on device (host does the final f64 reduce + divide):
  sum(b)        : ones^T @ b matmuls accumulated in PSUM on the idle PE
  sum(b * lse)  : lse*b == ln((s2-1)*b + 1) with s2 = e0+e1+e2  -> ACT ln with
                  bias=1 and accum_out (exact: b=0 -> ln(1)=0)
  sum(b * xsel) : sum_ch (Lb == ch+4) * x_ch  with Lb = L + 4*b, via
                  scalar_tensor_tensor accum (mask only fires when b=1)

Engine cost model (measured): plain tensor_tensor bf16 runs the 2x packed
mode (~0.71 cyc/elem), scalar_tensor_tensor is always 1x (~1.24 cyc/elem),
tensor_scalar single-src reaches 4x, ACT is ~1 elem/cyc.  gpsimd elementwise
and SWDGE accum-DMAs are net losses here (slow Q7 ucode + shared-SBUF-port
contention with the DVE; SWDGE accum also hits runtime INTERNAL errors under
load), so everything lives on DVE/ACT/PE/HWDGE-DMA.

Sharding: pure data parallel, 4 images per core.  Labels are shipped as int8
(low byte of the int32/int64 input - pure layout slicing) in a host-built
halo layout [128, 8, 772]: partition p holds image rows 6p-1..6p+6 (edge
clamped) and cols -2..769 (edge clamped), so every neighbour read is a plain
in-partition AP shift - no device seam copies.
"""

import numpy as np

B, C, H, W = 32, 3, 768, 768
NCORES = 8
BLOC = B // NCORES   # images per core
P = 128
TPB = H // P         # rows per partition (6)
HR = TPB + 2         # halo rows per partition (8)
HWID = W + 4         # halo width (772): col k <-> orig col k-2
WCH = 2              # t-blocks per compute chunk
NCH = TPB // WCH     # chunks per image (3)
CHW = WCH * W        # elems per chunk per partition (1536)
NCOL = 64            # accumulator columns (4 b + 12 ln + 36 xsel, rest zero)

_CACHE = {}


def _build():
    import concourse.bacc as bacc
    import concourse.tile as tile
    import concourse.mybir as mybir

    fp32 = mybir.dt.float32
    bf16 = mybir.dt.bfloat16
    i8 = mybir.dt.int8
    Alu = mybir.AluOpType
    Act = mybir.ActivationFunctionType

    nc = bacc.Bacc(
        "TRN2",
        target_bir_lowering=False,
        debug=False,
        enable_asserts=False,
        num_devices=NCORES,
    )
    preds = nc.dram_tensor(
        "preds", [BLOC, C, P, TPB * W], fp32, kind="ExternalInput"
    ).ap()
    labs = nc.dram_tensor(
        "labs", [BLOC, P, HR, HWID], i8, kind="ExternalInput"
    ).ap()
    outp = nc.dram_tensor("partials", [P, NCOL], fp32, kind="ExternalOutput").ap()
    outb = nc.dram_tensor("bsums", [1, 384], fp32, kind="ExternalOutput").ap()

    MMW = 384  # ones-matmul block width for sum(b) on the PE
    with tile.TileContext(nc) as tc:
        with (
            tc.tile_pool(name="ps", bufs=1, space="PSUM") as ps_pool,
            tc.tile_pool(name="lab", bufs=1) as lab_pool,
            tc.tile_pool(name="lfe", bufs=1) as lfe_pool,
            tc.tile_pool(name="cc", bufs=1) as c_pool,
            tc.tile_pool(name="bb", bufs=2) as b_pool,
            tc.tile_pool(name="lb", bufs=2) as lb_pool,
            tc.tile_pool(name="xin", bufs=2) as x_pool,
            tc.tile_pool(name="eact", bufs=1) as e_pool,
            tc.tile_pool(name="sls", bufs=1) as s_pool,
            tc.tile_pool(name="jnk", bufs=1) as j_pool,
            tc.tile_pool(name="accp", bufs=1) as accp,
        ):
            cols = accp.tile([P, NCOL], fp32, name="cols")
            nc.vector.memset(cols[:], 0.0)
            ones = accp.tile([P, 1], bf16, name="ones")
            nc.vector.memset(ones[:], 1.0)
            pb = ps_pool.tile([1, MMW], fp32, name="pb")
            junk1 = j_pool.tile([P, TPB, W], bf16, name="junk1")
            junk2 = j_pool.tile([P, WCH, W], bf16, name="junk2")
            for b in range(BLOC):
                labs_t = lab_pool.tile([P, HR, HWID], i8, name="labs_t", tag="labs_t")
                nc.sync.dma_start(out=labs_t[:], in_=labs[b])
                # int8 {0,1,2} -> bf16 casts.  Lf holds cols -2..769 (col k <->
                # orig k-2); Lfo holds the same data shifted one column (col k
                # <-> orig k-1) so the odd-column neighbour reads in nx stay
                # 4-byte aligned and the DVE keeps its 2x bf16 mode.
                Lf = lfe_pool.tile([P, HR, HWID], bf16, name="Lf", tag="Lf")
                nc.scalar.activation(Lf[:], labs_t[:], Act.Copy)
                # Lfo = labels shifted one column (keeps nx operands 4-byte
                # aligned for the DVE 2x bf16 mode)
                Lfo = lfe_pool.tile([P, TPB, W + 2], bf16, name="Lfo", tag="Lfo")
                nc.scalar.activation(
                    Lfo[:], labs_t[:, 1 : TPB + 1, 1 : W + 3], Act.Copy
                )
                # interior pixel (t, j), t in 0..5, j in 0..767 <-> Lf[t+1, j+2]
                nx = c_pool.tile([P, TPB, W], bf16, name="nx", tag="nx")
                nc.vector.tensor_tensor(
                    nx[:], Lfo[:, :, 0:W], Lfo[:, :, 2 : W + 2], Alu.not_equal
                )
                ny = c_pool.tile([P, TPB, W], bf16, name="ny", tag="ny")
                nc.vector.tensor_tensor(
                    ny[:], Lf[:, 0:TPB, 2 : W + 2], Lf[:, 2:HR, 2 : W + 2],
                    Alu.not_equal,
                )
                b_t = b_pool.tile([P, TPB, W], bf16, name="b_t", tag="b_t")
                nc.vector.tensor_tensor(b_t[:], nx[:], ny[:], Alu.max)
                # sum(b) on the idle PE: ones^T @ b blocks accumulated in PSUM
                for t in range(TPB):
                    for h in range(W // MMW):
                        nc.tensor.matmul(
                            pb[:, :],
                            ones[:],
                            b_t[:, t, h * MMW : (h + 1) * MMW],
                            start=(b == 0 and t == 0 and h == 0),
                            stop=(b == BLOC - 1 and t == TPB - 1 and h == W // MMW - 1),
                        )
                # Lb = 4*b + L : {0,1,2} off-boundary, {4,5,6} on-boundary
                b4 = c_pool.tile([P, TPB, W], bf16, name="b4", tag="b4")
                nc.vector.tensor_scalar(b4[:], b_t[:], 4.0, None, Alu.mult)
                Lb = lb_pool.tile([P, TPB, W], bf16, name="Lb", tag="Lb")
                nc.vector.tensor_add(Lb[:], b4[:], Lf[:, 1 : TPB + 1, 2 : W + 2])
                # exps land in image-level E tiles (per-chunk slices); the
                # whole softmax-sum path then runs at [P, 4608] granularity
                # to amortize per-instruction overhead
                E = [
                    e_pool.tile([P, TPB, W], bf16, name=f"E{ch}", tag=f"E{ch}")
                    for ch in range(C)
                ]
                for c in range(NCH):
                    t0 = WCH * c
                    xs = []
                    for ch in range(C):
                        x = x_pool.tile([P, WCH, W], fp32, name=f"x{ch}", tag=f"x{ch}")
                        nc.sync.dma_start(
                            out=x[:], in_=preds[b, ch, :, t0 * W : (t0 + WCH) * W]
                        )
                        xs.append(x)
                    for ch in range(C):
                        nc.scalar.activation(
                            E[ch][:, t0 : t0 + WCH, :], xs[ch][:], Act.Exp
                        )
                    # sum(b * x[label]) via masked accumulate, per channel
                    for ch in range(C):
                        xcol = BLOC + BLOC * NCH + (b * NCH + c) * C + ch
                        nc.vector.scalar_tensor_tensor(
                            junk2[:],
                            Lb[:, t0 : t0 + WCH, :],
                            float(ch + 4),
                            xs[ch][:],
                            Alu.is_equal,
                            Alu.mult,
                            accum_out=cols[:, xcol : xcol + 1],
                        )
                # s2m1 = e0+e1+e2-1 via (e2-1) on the 4x single-src path
                s1 = s_pool.tile([P, TPB, W], bf16, name="s1", tag="s1")
                nc.vector.tensor_add(s1[:], E[0][:], E[1][:])
                e2m1 = s_pool.tile([P, TPB, W], bf16, name="e2m1", tag="e2m1")
                nc.vector.tensor_scalar(e2m1[:], E[2][:], 1.0, None, Alu.subtract)
                s2m1 = s_pool.tile([P, TPB, W], bf16, name="s2m1", tag="s2m1")
                nc.vector.tensor_add(s2m1[:], s1[:], e2m1[:])
                # t1 = (s2-1)*b ; ln(t1+1) = lse*b, accumulated on ACT
                t1 = s_pool.tile([P, TPB, W], bf16, name="t1", tag="t1")
                nc.vector.tensor_mul(t1[:], s2m1[:], b_t[:])
                lncol = BLOC + b
                nc.scalar.activation(
                    junk1[:], t1[:], Act.Ln, bias=1.0, scale=1.0,
                    accum_out=cols[:, lncol : lncol + 1],
                )
            sb = accp.tile([1, MMW], fp32, name="sb")
            nc.vector.tensor_copy(sb[:], pb[:])
            nc.sync.dma_start(out=outp[:, :], in_=cols[:])
            nc.sync.dma_start(out=outb[:, :], in_=sb[:])

    # Pin Exp/Ln to the one table set containing both so the ACT table loads
    # once instead of thrashing between sets.
    from concourse import hw_specs

    KEEP = "natural_log_exp_and_others"
    orig = hw_specs.get_activation_tables

    def only_combined(arch):
        t = orig(arch)
        return {name: (funcs if name == KEEP else set()) for name, funcs in t.items()}

    patched = []
    for mod in (hw_specs, bacc):
        if getattr(mod, "get_activation_tables", None) is not None:
            patched.append((mod, mod.get_activation_tables))
            mod.get_activation_tables = only_combined
    try:
        nc.compile()
    finally:
        for mod, fn in patched:
            mod.get_activation_tables = fn
    return nc


def _get_nc():
    if "nc" not in _CACHE:
        _CACHE["nc"] = _build()
    return _CACHE["nc"]


def _prep_inputs(predictions, labels):
    preds = np.ascontiguousarray(predictions, dtype=np.float32).reshape(
        NCORES, BLOC, C, P, TPB * W
    )
    labels = np.ascontiguousarray(labels)
    it = labels.dtype.itemsize
    # low byte of each little-endian int element == the label value (0..2)
    lab8 = labels.view(np.int8).reshape(B, H, W, it)[..., 0]
    padded = np.empty((B, H + 2, HWID), dtype=np.int8)
    padded[:, 1 : H + 1, 2 : W + 2] = lab8
    padded[:, 0, :] = 0
    padded[:, H + 1, :] = 0
    padded[:, 0, 2 : W + 2] = lab8[:, 0, :]      # row -1 := row 0
    padded[:, H + 1, 2 : W + 2] = lab8[:, -1, :]  # row 768 := row 767
    padded[:, :, 1] = padded[:, :, 2]             # col -1 := col 0
    padded[:, :, 0] = padded[:, :, 2]
    padded[:, :, W + 2] = padded[:, :, W + 1]     # col 768 := col 767
    padded[:, :, W + 3] = padded[:, :, W + 1]
    s = padded.strides
    halo = np.lib.stride_tricks.as_strided(
        padded, shape=(B, P, HR, HWID), strides=(s[0], TPB * s[1], s[1], s[2])
    )
    labs = np.ascontiguousarray(halo).reshape(NCORES, BLOC, P, HR, HWID)
    return preds, labs


def _host_reduce(partials, bsums):
    tot_b = 0.0
    tot_lse = 0.0
    tot_x = 0.0
    for r, bs in zip(partials, bsums):
        p = r.astype(np.float64)
        tot_b += bs.astype(np.float64).sum()
        tot_lse += p[:, BLOC : BLOC + BLOC * NCH].sum()
        tot_x += p[:, BLOC + BLOC * NCH : BLOC + BLOC * NCH + BLOC * NCH * C].sum()
    return np.float32((tot_lse - tot_x) / (tot_b + 1e-8))


def kernel(predictions, labels):
    from concourse.bass_utils import run_bass_kernel_spmd

    preds, labs = _prep_inputs(predictions, labels)
    nc = _get_nc()
    in_maps = [{"preds": preds[i], "labs": labs[i]} for i in range(NCORES)]
    res = run_bass_kernel_spmd(nc, in_maps, list(range(NCORES))).results
    return _host_reduce(
        [r["partials"] for r in res], [r["bsums"] for r in res]
    )


# revision 31
# speedup vs baseline: 1.0017x; 1.0017x over previous
"""BoundaryLoss kernel for 8 Trainium2 NeuronCores.

Math (equivalent to the reference):
  boundary b(i,j) = [L(i,j-1) != L(i,j+1)]_clamped OR [L(i-1,j) != L(i+1,j)]_clamped
    (union of class-1/2 indicator boundaries == "any label change", since any
     differing pair in {0,1,2} differs in membership of class 1 or class 2)
  ce  = logsumexp_c(x) - x[label]
  loss = sum(ce * b) / (sum(b) + 1e-8)

Decomposition used on device (host does the final f64 reduce + divide):
  sum(b)        : ones^T @ b matmuls accumulated in PSUM on the idle PE
  sum(b * lse)  : lse*b == ln((s2-1)*b + 1) with s2 = e0+e1+e2  -> ACT ln with
                  bias=1 and accum_out (exact: b=0 -> ln(1)=0)
  sum(b * xsel) : sum_ch (Lb == ch+4) * x_ch  with Lb = L + 4*b, via
                  scalar_tensor_tensor accum (mask only fires when b=1)

Engine cost model (measured): plain tensor_tensor bf16 runs the 2x packed
mode (~0.71 cyc/elem), scalar_tensor_tensor is always 1x (~1.24 cyc/elem),
tensor_scalar single-src reaches 4x, ACT is ~1 elem/cyc.  gpsimd elementwise
and SWDGE accum-DMAs are net losses here (slow Q7 ucode + shared-SBUF-port
contention with the DVE; SWDGE accum also hits runtime INTERNAL errors under
load), so everything lives on DVE/ACT/PE/HWDGE-DMA.

Sharding: pure data parallel, 4 images per core.  Labels are shipped as int8
(low byte of the int32/int64 input - pure layout slicing) in a host-built
halo layout [128, 8, 772]: partition p holds image rows 6p-1..6p+6 (edge
clamped) and cols -2..769 (edge clamped), so every neighbour read is a plain
in-partition AP shift - no device seam copies.
"""

import numpy as np

B, C, H, W = 32, 3, 768, 768
NCORES = 8
BLOC = B // NCORES   # images per core
P = 128
TPB = H // P         # rows per partition (6)
HR = TPB + 2         # halo rows per partition (8)
HWID = W + 4         # halo width (772): col k <-> orig col k-2
WCH = 2              # t-blocks per compute chunk
NCH = TPB // WCH     # chunks per image (3)
CHW = WCH * W        # elems per chunk per partition (1536)
NCOL = 64            # accumulator columns (4 b + 12 ln + 36 xsel, rest zero)

_CACHE = {}


def _build():
    import concourse.bacc as bacc
    import concourse.tile as tile
    import concourse.mybir as mybir

    fp32 = mybir.dt.float32
    bf16 = mybir.dt.bfloat16
    i8 = mybir.dt.int8
    Alu = mybir.AluOpType
    Act = mybir.ActivationFunctionType

    nc = bacc.Bacc(
        "TRN2",
        target_bir_lowering=False,
        debug=False,
        enable_asserts=False,
        num_devices=NCORES,
    )
    preds = nc.dram_tensor(
        "preds", [BLOC, C, P, TPB * W], fp32, kind="ExternalInput"
    ).ap()
    labs = nc.dram_tensor(
        "labs", [BLOC, P, HR, HWID], i8, kind="ExternalInput"
    ).ap()
    outp = nc.dram_tensor("partials", [P, NCOL], fp32, kind="ExternalOutput").ap()
    outb = nc.dram_tensor("bsums", [1, 384], fp32, kind="ExternalOutput").ap()

    MMW = 384  # ones-matmul block width for sum(b) on the PE
    with tile.TileContext(nc) as tc:
        with (
            tc.tile_pool(name="ps", bufs=1, space="PSUM") as ps_pool,
            tc.tile_pool(name="lab", bufs=2) as lab_pool,
            tc.tile_pool(name="lfe", bufs=1) as lfe_pool,
            tc.tile_pool(name="cc", bufs=1) as c_pool,
            tc.tile_pool(name="bb", bufs=2) as b_pool,
            tc.tile_pool(name="lb", bufs=2) as lb_pool,
            tc.tile_pool(name="xin", bufs=2) as x_pool,
            tc.tile_pool(name="eact", bufs=1) as e_pool,
            tc.tile_pool(name="sls", bufs=1) as s_pool,
            tc.tile_pool(name="jnk", bufs=1) as j_pool,
            tc.tile_pool(name="accp", bufs=1) as accp,
        ):
            cols = accp.tile([P, NCOL], fp32, name="cols")
            nc.vector.memset(cols[:], 0.0)
            ones = accp.tile([P, 1], bf16, name="ones")
            nc.vector.memset(ones[:], 1.0)
            pb = ps_pool.tile([1, MMW], fp32, name="pb")
            junk1 = j_pool.tile([P, TPB, W], bf16, name="junk1")
            for b in range(BLOC):
                labs_t = lab_pool.tile([P, HR, HWID], i8, name="labs_t", tag="labs_t")
                nc.sync.dma_start(out=labs_t[:], in_=labs[b])
                # int8 {0,1,2} -> bf16 casts.  Lf holds cols -2..769 (col k <->
                # orig k-2); Lfo holds the same data shifted one column (col k
                # <-> orig k-1) so the odd-column neighbour reads in nx stay
                # 4-byte aligned and the DVE keeps its 2x bf16 mode.
                Lf = lfe_pool.tile([P, HR, HWID], bf16, name="Lf", tag="Lf")
                nc.scalar.activation(Lf[:], labs_t[:], Act.Copy)
                # Lfo = labels shifted one column (keeps nx operands 4-byte
                # aligned for the DVE 2x bf16 mode)
                Lfo = lfe_pool.tile([P, TPB, W + 2], bf16, name="Lfo", tag="Lfo")
                nc.scalar.activation(
                    Lfo[:], labs_t[:, 1 : TPB + 1, 1 : W + 3], Act.Copy
                )
                # interior pixel (t, j), t in 0..5, j in 0..767 <-> Lf[t+1, j+2]
                nx = c_pool.tile([P, TPB, W], bf16, name="nx", tag="nx")
                nc.vector.tensor_tensor(
                    nx[:], Lfo[:, :, 0:W], Lfo[:, :, 2 : W + 2], Alu.not_equal
                )
                ny = c_pool.tile([P, TPB, W], bf16, name="ny", tag="ny")
                nc.vector.tensor_tensor(
                    ny[:], Lf[:, 0:TPB, 2 : W + 2], Lf[:, 2:HR, 2 : W + 2],
                    Alu.not_equal,
                )
                b_t = b_pool.tile([P, TPB, W], bf16, name="b_t", tag="b_t")
                nc.vector.tensor_tensor(b_t[:], nx[:], ny[:], Alu.max)
                # sum(b) on the idle PE: ones^T @ b blocks accumulated in PSUM
                for t in range(TPB):
                    for h in range(W // MMW):
                        nc.tensor.matmul(
                            pb[:, :],
                            ones[:],
                            b_t[:, t, h * MMW : (h + 1) * MMW],
                            start=(b == 0 and t == 0 and h == 0),
                            stop=(b == BLOC - 1 and t == TPB - 1 and h == W // MMW - 1),
                        )
                # Lb = 4*b + L : {0,1,2} off-boundary, {4,5,6} on-boundary
                # (4*b scratch reuses the dead nx tile)
                nc.vector.tensor_scalar(nx[:], b_t[:], 4.0, None, Alu.mult)
                Lb = lb_pool.tile([P, TPB, W], bf16, name="Lb", tag="Lb")
                nc.vector.tensor_add(Lb[:], nx[:], Lf[:, 1 : TPB + 1, 2 : W + 2])
                # exps land in image-level E tiles (per-chunk slices); the
                # whole softmax-sum path then runs at [P, 4608] granularity
                # to amortize per-instruction overhead
                E = [
                    e_pool.tile([P, TPB, W], bf16, name=f"E{ch}", tag=f"E{ch}")
                    for ch in range(C)
                ]
                for c in range(NCH):
                    t0 = WCH * c
                    xs = []
                    for ch in range(C):
                        x = x_pool.tile([P, WCH, W], fp32, name=f"x{ch}", tag=f"x{ch}")
                        nc.sync.dma_start(
                            out=x[:], in_=preds[b, ch, :, t0 * W : (t0 + WCH) * W]
                        )
                        xs.append(x)
                    for ch in range(C):
                        nc.scalar.activation(
                            E[ch][:, t0 : t0 + WCH, :], xs[ch][:], Act.Exp
                        )
                    # sum(b * x[label]) via masked accumulate, per channel
                    for ch in range(C):
                        xcol = BLOC + BLOC * NCH + (b * NCH + c) * C + ch
                        nc.vector.scalar_tensor_tensor(
                            junk1[:, 0:WCH, :],
                            Lb[:, t0 : t0 + WCH, :],
                            float(ch + 4),
                            xs[ch][:],
                            Alu.is_equal,
                            Alu.mult,
                            accum_out=cols[:, xcol : xcol + 1],
                        )
                # s2m1 = e0+e1+e2-1 via (e2-1) on the 4x single-src path
                s1 = s_pool.tile([P, TPB, W], bf16, name="s1", tag="s1")
                nc.vector.tensor_add(s1[:], E[0][:], E[1][:])
                e2m1 = s_pool.tile([P, TPB, W], bf16, name="e2m1", tag="e2m1")
                nc.vector.tensor_scalar(e2m1[:], E[2][:], 1.0, None, Alu.subtract)
                s2m1 = s_pool.tile([P, TPB, W], bf16, name="s2m1", tag="s2m1")
                nc.vector.tensor_add(s2m1[:], s1[:], e2m1[:])
                # t1 = (s2-1)*b ; ln(t1+1) = lse*b, accumulated on ACT
                t1 = s_pool.tile([P, TPB, W], bf16, name="t1", tag="t1")
                nc.vector.tensor_mul(t1[:], s2m1[:], b_t[:])
                lncol = BLOC + b
                nc.scalar.activation(
                    junk1[:], t1[:], Act.Ln, bias=1.0, scale=1.0,
                    accum_out=cols[:, lncol : lncol + 1],
                )
            sb = accp.tile([1, MMW], fp32, name="sb")
            nc.vector.tensor_copy(sb[:], pb[:])
            nc.sync.dma_start(out=outp[:, :], in_=cols[:])
            nc.sync.dma_start(out=outb[:, :], in_=sb[:])

    # Pin Exp/Ln to the one table set containing both so the ACT table loads
    # once instead of thrashing between sets.
    from concourse import hw_specs

    KEEP = "natural_log_exp_and_others"
    orig = hw_specs.get_activation_tables

    def only_combined(arch):
        t = orig(arch)
        return {name: (funcs if name == KEEP else set()) for name, funcs in t.items()}

    patched = []
    for mod in (hw_specs, bacc):
        if getattr(mod, "get_activation_tables", None) is not None:
            patched.append((mod, mod.get_activation_tables))
            mod.get_activation_tables = only_combined
    try:
        nc.compile()
    finally:
        for mod, fn in patched:
            mod.get_activation_tables = fn
    return nc


def _get_nc():
    if "nc" not in _CACHE:
        _CACHE["nc"] = _build()
    return _CACHE["nc"]


def _prep_inputs(predictions, labels):
    preds = np.ascontiguousarray(predictions, dtype=np.float32).reshape(
        NCORES, BLOC, C, P, TPB * W
    )
    labels = np.ascontiguousarray(labels)
    it = labels.dtype.itemsize
    # low byte of each little-endian int element == the label value (0..2)
    lab8 = labels.view(np.int8).reshape(B, H, W, it)[..., 0]
    padded = np.empty((B, H + 2, HWID), dtype=np.int8)
    padded[:, 1 : H + 1, 2 : W + 2] = lab8
    padded[:, 0, :] = 0
    padded[:, H + 1, :] = 0
    padded[:, 0, 2 : W + 2] = lab8[:, 0, :]      # row -1 := row 0
    padded[:, H + 1, 2 : W + 2] = lab8[:, -1, :]  # row 768 := row 767
    padded[:, :, 1] = padded[:, :, 2]             # col -1 := col 0
    padded[:, :, 0] = padded[:, :, 2]
    padded[:, :, W + 2] = padded[:, :, W + 1]     # col 768 := col 767
    padded[:, :, W + 3] = padded[:, :, W + 1]
    s = padded.strides
    halo = np.lib.stride_tricks.as_strided(
        padded, shape=(B, P, HR, HWID), strides=(s[0], TPB * s[1], s[1], s[2])
    )
    labs = np.ascontiguousarray(halo).reshape(NCORES, BLOC, P, HR, HWID)
    return preds, labs


def _host_reduce(partials, bsums):
    tot_b = 0.0
    tot_lse = 0.0
    tot_x = 0.0
    for r, bs in zip(partials, bsums):
        p = r.astype(np.float64)
        tot_b += bs.astype(np.float64).sum()
        tot_lse += p[:, BLOC : BLOC + BLOC * NCH].sum()
        tot_x += p[:, BLOC + BLOC * NCH : BLOC + BLOC * NCH + BLOC * NCH * C].sum()
    return np.float32((tot_lse - tot_x) / (tot_b + 1e-8))


def kernel(predictions, labels):
    from concourse.bass_utils import run_bass_kernel_spmd

    preds, labs = _prep_inputs(predictions, labels)
    nc = _get_nc()
    in_maps = [{"preds": preds[i], "labs": labs[i]} for i in range(NCORES)]
    res = run_bass_kernel_spmd(nc, in_maps, list(range(NCORES))).results
    return _host_reduce(
        [r["partials"] for r in res], [r["bsums"] for r in res]
    )
